# revision 1
# baseline (speedup 1.0000x reference)
"""Trainium2 Bass kernel for nn_Attention (b=4, n=2048, d=1024, 16 heads x 64).

Sharding: 8 cores = 4 batches x 2 head-groups (8 heads each).

Per core (transposed-layout pipeline, no intermediate transposes):
  A: x^T via XBAR DMA-transpose (bf16)
  B: q^T/k^T = (x @ w_qk)^T per head-pair (bf16 matmuls, fp32 psum)
  C: v = x @ w_v with a ones column appended per head
  D: scores^T = K @ Q^T (row-tiled K=64 pairs) -> exp (ACT, fp32->bf16)
     -> av^T = [V|1]^T @ exp^T, giving the softmax denominator for free;
     normalize with DVE reciprocal + gpsimd partition broadcast
  E: out = av @ w_proj + bias in float32r (accuracy-critical last layer)

dtype choices are empirical: bf16 matmul ~213ns/512-col vs ~1us for f32r;
ACT exp fp32->bf16 runs at full rate (396ns) vs 2.5us for fp32->fp32.
bf16 noise in scores/attention is suppressed by softmax normalization
(common mode) and diffuse averaging over 2048 keys; the final projection
stays f32r because its error passes straight through.

Host side: shards inputs (bf16 casts, q-scale folded into w_q), feeds 8
cores via PJRT/axon, sums the two head-group partials per batch.
"""
import sys

sys.path.insert(0, "/opt/trn_rl_repo")

import ml_dtypes
import numpy as np

import concourse.bass as bass
import concourse.mybir as mybir
import concourse.tile as tile
from concourse import bacc
from concourse.bass import ts, ds

F32 = mybir.dt.float32
F32R = mybir.dt.float32r
BF16 = mybir.dt.bfloat16
FP16 = mybir.dt.float16
AF = mybir.ActivationFunctionType

SEQ = 2048
DIM = 1024
H = 8  # heads per core
HD = 64
QK = 1024  # q cols (512) ++ k cols (512) per core
VC = 512  # v cols per core
E = 1024  # output dim
KSUB = DIM // 128  # 8
ITILE = 512
NIT = SEQ // ITILE  # 4
NJS = SEQ // 128  # 16
NHP = H // 2  # 4 head-pairs


def build_attention(iters: int = 1, stages: int = 5):
    nc = bacc.Bacc("TRN2", target_bir_lowering=False, debug=False)
    x = nc.dram_tensor("x", [SEQ, DIM], FP16, kind="ExternalInput")
    w_qk = nc.dram_tensor("w_qk", [DIM, QK], FP16, kind="ExternalInput")
    w_v = nc.dram_tensor("w_v", [DIM, VC], FP16, kind="ExternalInput")
    w_proj = nc.dram_tensor("w_proj", [VC, E], F32, kind="ExternalInput")
    bias = nc.dram_tensor("bias", [E], F32, kind="ExternalInput")
    out = nc.dram_tensor("out", [SEQ, E], F32, kind="ExternalOutput")

    w_qk_r = w_qk.rearrange("(ko p) c -> p ko c", p=128)  # [128, 8, 1024]
    w_v_r = w_v.rearrange("(ko p) c -> p ko c", p=128)  # [128, 8, 512]
    w_proj_r = w_proj.rearrange("(cs p) e -> p cs e", p=128)  # [128, 4, 1024]

    with tile.TileContext(nc) as tc:
        with (
            tc.tile_pool(name="cpool", bufs=1) as cpool,
            tc.tile_pool(name="qkring", bufs=3) as qkring,
            tc.tile_pool(name="stream", bufs=3) as stream,
            tc.tile_pool(name="epool", bufs=8) as epool,
            tc.tile_pool(name="npool", bufs=4) as npool,
            tc.tile_pool(name="opool", bufs=3) as opool,
            tc.tile_pool(name="psum", bufs=2, space="PSUM") as psum,
            tc.tile_pool(name="psum4", bufs=2, space="PSUM") as psum4,
        ):
            pools = (cpool, qkring, stream, epool, npool, opool, psum, psum4)
            if iters == 1:
                one_iter(tc, nc, x, w_qk_r, w_v_r, w_proj_r, bias, out, pools, stages)
            else:
                with tc.For_i(0, iters, 1):
                    one_iter(
                        tc, nc, x, w_qk_r, w_v_r, w_proj_r, bias, out, pools, stages
                    )
    nc.compile()
    return nc


def one_iter(tc, nc, x, w_qk_r, w_v_r, w_proj_r, bias, out, pools, stages=5):
    cpool, qkring, stream, epool, npool, opool, psum, psum4 = pools

    v_sb = cpool.tile([128, NJS, H * (HD + 1)], FP16, tag="v")  # per head 65 cols
    v_view = v_sb[:].rearrange("p j (h c) -> p j h c", c=HD + 1)
    # fill with ones via broadcast DMA; stage C overwrites the V columns,
    # leaving the per-head ones column (index HD) for the softmax denominator
    ones_dram = nc.inline_tensor(
        np.ones((NJS, H * (HD + 1)), np.float16 if FP16 == mybir.dt.float16 else ml_dtypes.bfloat16),
        "ones_fill",
    )
    nc.sync.dma_start(
        v_sb[:], ones_dram.ap()[None, :, :].to_broadcast((128, NJS, H * (HD + 1)))
    )
    avT = cpool.tile([128, NHP, SEQ], F32R, tag="avT")
    xT = cpool.tile([128, KSUB, SEQ], FP16, tag="xT")
    w_v_sb = cpool.tile([128, KSUB, VC], FP16, tag="wv")
    nc.sync.dma_start(w_v_sb[:], w_v_r[:])
    wproj_sb = cpool.tile([128, VC // 128, E], F32R, tag="wproj")
    nc.sync.dma_start(wproj_sb[:], w_proj_r[:].bitcast(F32R))
    bias_rep = cpool.tile([128, E], F32, tag="bias")
    nc.sync.dma_start(bias_rep[:], bias[None, :].to_broadcast((128, E)))

    out_r = out.rearrange("(p a) e -> p (a e)", p=128)

    # ---- Stage A: x^T via XBAR DMA transpose (fp16), split per 512-block
    # so stages C/B can start as soon as the first seq block is transposed
    for ib in range(SEQ // 512):
        for ksv in range(KSUB):
            nc.sync.dma_start_transpose(
                xT[:, ksv, ts(ib, 512)], x[ts(ib, 512), ts(ksv, 128)]
            )

    if stages <= 1:
        nc.sync.dma_start(
            out_r[:].bitcast(FP16)[:, 0 : KSUB * SEQ],
            xT[:].rearrange("p k s -> p (k s)"),
        )
        return

    # ---- Stage C: v = x @ w_v; psum-bank alternation over jt pairs ----
    for jt2 in range(NJS // 2):
        pss = [
            psum.tile([128, VC], F32, tag="g", name=f"psv{i}") for i in range(2)
        ]
        for ksv in range(KSUB):
            for i in range(2):
                nc.tensor.matmul(
                    pss[i][:],
                    xT[:, ksv, ts(2 * jt2 + i, 128)],
                    w_v_sb[:, ksv, :],
                    start=(ksv == 0),
                    stop=(ksv == KSUB - 1),
                )
        for i in range(2):
            nc.vector.tensor_copy(
                v_view[:, 2 * jt2 + i, :, 0:HD],
                pss[i][:].rearrange("p (h c) -> p h c", c=HD),
            )

    if stages <= 2:
        nc.sync.dma_start(
            out_r[:].bitcast(FP16)[:, 0 : NJS * H * (HD + 1)],
            v_sb[:].rearrange("p j c -> p (j c)"),
        )
        return

    # ---- per head-pair: B(hp) then D(hp) ----
    for hp in range(NHP):
        # B: q^T and k^T for this pair; it-pair bank alternation
        qTh = qkring.tile([128, SEQ], FP16, tag="qT", name=f"qT{hp}")
        kTh = qkring.tile([128, SEQ], FP16, tag="kT", name=f"kT{hp}")
        for ct, dest in ((hp, qTh), (hp + 4, kTh)):
            w_t = stream.tile([128, KSUB, 128], FP16, tag="wqk")
            nc.sync.dma_start(w_t[:], w_qk_r[:, :, ts(ct, 128)])
            for it2 in range(NIT // 2):
                pss = [
                    psum.tile([128, ITILE], F32, tag="g", name=f"psb{i}")
                    for i in range(2)
                ]
                for ksv in range(KSUB):
                    for i in range(2):
                        nc.tensor.matmul(
                            pss[i][:],
                            w_t[:, ksv, :],
                            xT[:, ksv, ts(2 * it2 + i, ITILE)],
                            start=(ksv == 0),
                            stop=(ksv == KSUB - 1),
                        )
                for i in range(2):
                    nc.vector.tensor_copy(dest[:, ts(2 * it2 + i, ITILE)], pss[i][:])

        if stages <= 3:
            nc.sync.dma_start(
                out_r[:, ds(hp * 4096, SEQ)].bitcast(FP16)[:, 0:SEQ], qTh[:]
            )
            nc.sync.dma_start(
                out_r[:, ds(hp * 4096 + SEQ, SEQ)].bitcast(FP16)[:, 0:SEQ], kTh[:]
            )
            continue

        # D: attention; paired scores/exp over js pairs, AV lags one pair
        for it in range(NIT):
            av_ps = [
                psum4.tile([HD + 1, ITILE], F32, tag="av", name=f"av{h01}")
                for h01 in range(2)
            ]

            def emit_av(jsp, es):
                for h01 in range(2):
                    for half in range(2):
                        nc.tensor.matmul(
                            av_ps[h01][:],
                            v_view[:, 2 * jsp + half, 2 * hp + h01, :],
                            es[h01][:, half, :],
                            start=(jsp == 0 and half == 0),
                            stop=(jsp == NJS // 2 - 1 and half == 1),
                        )

            pend = []
            for jsp in range(NJS // 2):
                cur = []
                for h01 in range(2):
                    sl = slice(h01 * 64, h01 * 64 + 64)
                    sp = psum.tile([128, 2, ITILE], F32, tag="s")
                    for half in range(2):
                        nc.tensor.matmul(
                            sp[:, half, :],
                            kTh[sl, ts(2 * jsp + half, 128)],
                            qTh[sl, ts(it, ITILE)],
                            start=True,
                            stop=True,
                        )
                    e = epool.tile([128, 2, ITILE], BF16, tag="e")
                    nc.scalar.activation(e[:], sp[:], AF.Exp)
                    cur.append(e)
                pend.append((jsp, cur))
                if len(pend) > 2:  # AV lags two exp-pairs behind
                    j0, es = pend.pop(0)
                    emit_av(j0, es)
            for j0, es in pend:
                emit_av(j0, es)

            for h01 in range(2):
                h = 2 * hp + h01
                # free the av psum bank with a single copy; normalize from
                # SBUF off the critical path (gpsimd broadcast is ~2.5us)
                avU = npool.tile([HD + 1, ITILE], F32, tag="avU")
                nc.vector.tensor_copy(avU[:], av_ps[h01][:])
                rc = npool.tile([1, ITILE], F32, tag="rc")
                nc.vector.reciprocal(rc[:], avU[HD : HD + 1, :])
                rr = npool.tile([64, ITILE], F32, tag="rr")
                nc.gpsimd.partition_broadcast(rr[:], rc[:])
                if h01 == 0:
                    nc.vector.tensor_mul(
                        avT[0:64, h // 2, ts(it, ITILE)],
                        avU[0:HD, :],
                        rr[:],
                    )
                else:
                    tmp = npool.tile([64, ITILE], F32R, tag="tmp")
                    nc.vector.tensor_mul(tmp[:], avU[0:HD, :], rr[:])
                    nc.sync.dma_start(avT[64:128, h // 2, ts(it, ITILE)], tmp[:])

    if stages <= 3:
        return
    if stages <= 4:
        nc.sync.dma_start(
            out_r[:, 0 : NHP * SEQ],
            avT[:].rearrange("p k s -> p (k s)").bitcast(F32),
        )
        return

    # ---- Stage E: out = avRow @ w_proj + bias (f32r); et-pair alternation ----
    for it in range(SEQ // 128):
        pss = [
            psum.tile([128, ITILE], F32, tag="s", name=f"pse{i}") for i in range(2)
        ]
        for cs in range(VC // 128):
            for et in range(2):
                nc.tensor.matmul(
                    pss[et][:],
                    avT[:, cs, ts(it, 128)],
                    wproj_sb[:, cs, ts(et, ITILE)],
                    start=(cs == 0),
                    stop=(cs == VC // 128 - 1),
                )
        for et in range(2):
            o = opool.tile([128, ITILE], F32, tag="o")
            nc.vector.tensor_add(o[:], pss[et][:], bias_rep[:, ts(et, ITILE)])
            nc.sync.dma_start(out[ts(it, 128), ts(et, ITILE)], o[:])


# ---------------- host side ----------------

_CACHE = {}


def _get_runner():
    if "runner" not in _CACHE:
        import jax
        from jax.sharding import Mesh, PartitionSpec
        from jax.experimental.shard_map import shard_map
        from concourse import bass2jax

        nc = build_attention(iters=1)
        bass2jax.install_neuronx_cc_hook()

        in_names, out_names, out_avals, zero_shapes = [], [], [], []
        partition_name = nc.partition_id_tensor.name if nc.partition_id_tensor else None
        for alloc in nc.m.functions[0].allocations:
            if not isinstance(alloc, mybir.MemoryLocationSet):
                continue
            name = alloc.memorylocations[0].name
            if alloc.kind == "ExternalInput":
                if name != partition_name:
                    in_names.append(name)
            elif alloc.kind == "ExternalOutput":
                out_names.append(name)
                shape = tuple(alloc.tensor_shape)
                dtype = mybir.dt.np(alloc.dtype)
                out_avals.append(jax.core.ShapedArray(shape, dtype))
                zero_shapes.append((shape, dtype))
        n_params = len(in_names)
        n_outs = len(out_avals)
        all_names = in_names + out_names
        if partition_name is not None:
            all_names = all_names + [partition_name]
        donate = tuple(range(n_params, n_params + n_outs))

        def _body(*args):
            operands = list(args)
            if partition_name is not None:
                operands.append(bass2jax.partition_id_tensor())
            outs = bass2jax._bass_exec_p.bind(
                *operands,
                out_avals=tuple(out_avals),
                in_names=tuple(all_names),
                out_names=tuple(out_names),
                lowering_input_output_aliases=(),
                sim_require_finite=True,
                sim_require_nnan=True,
                nc=nc,
            )
            return tuple(outs)

        devices = jax.devices()[:8]
        mesh = Mesh(np.asarray(devices), ("core",))
        in_specs = (PartitionSpec("core"),) * (n_params + n_outs)
        out_specs = (PartitionSpec("core"),) * n_outs
        sharded = jax.jit(
            shard_map(
                _body,
                mesh=mesh,
                in_specs=in_specs,
                out_specs=out_specs,
                check_rep=False,
            ),
            donate_argnums=donate,
            keep_unused=True,
        )
        _CACHE["runner"] = (sharded, in_names, out_names, out_avals, zero_shapes)
    return _CACHE["runner"]


def _shard_inputs(x, w_qkv, w_proj, b_proj):
    """Per-core input dicts. Core c: batch c//2, head-group c%2."""
    SCALE = HD**-0.5
    bf16 = np.float16
    in_maps = []
    zeros_bias = np.zeros_like(b_proj)
    for c in range(8):
        b = c // 2
        hg = c % 2
        qs = slice(hg * 512, (hg + 1) * 512)
        ks = slice(1024 + hg * 512, 1024 + (hg + 1) * 512)
        vs = slice(2048 + hg * 512, 2048 + (hg + 1) * 512)
        w_qk_c = np.concatenate(
            [w_qkv[:, qs] * np.float32(SCALE), w_qkv[:, ks]], axis=1
        ).astype(bf16)
        in_maps.append(
            {
                "x": x[b].astype(bf16),
                "w_qk": w_qk_c,
                "w_v": w_qkv[:, vs].astype(bf16),
                "w_proj": np.ascontiguousarray(w_proj[hg * 512 : (hg + 1) * 512]),
                "bias": b_proj if hg == 0 else zeros_bias,
            }
        )
    return in_maps


def kernel(x, w_qkv, w_proj, b_proj):
    import jax
    import jax.numpy as jnp

    x = np.asarray(x, dtype=np.float32)
    w_qkv = np.asarray(w_qkv, dtype=np.float32)
    w_proj = np.asarray(w_proj, dtype=np.float32)
    b_proj = np.asarray(b_proj, dtype=np.float32)

    sharded, in_names, out_names, out_avals, zero_shapes = _get_runner()
    in_maps = _shard_inputs(x, w_qkv, w_proj, b_proj)
    concat_in = [
        np.concatenate([in_maps[c][name] for c in range(8)], axis=0)
        for name in in_names
    ]
    zeros = [jnp.zeros((8 * s[0], *s[1:]), dt) for (s, dt) in zero_shapes]
    outs = sharded(*concat_in, *zeros)
    out_np = np.asarray(outs[out_names.index("out")]).reshape(8, SEQ, E)
    full = np.empty((4, SEQ, E), dtype=np.float32)
    for b in range(4):
        full[b] = out_np[2 * b] + out_np[2 * b + 1]
    return full



# revision 13
# speedup vs baseline: 27.1579x; 27.1579x over previous
"""Trainium2 Bass kernel for nn_Attention (b=4, n=2048, d=1024, 16 heads x 64).

Sharding: 8 cores = 4 batches x 2 head-groups (8 heads each); host sums the
two head-group partials per batch (proj is split along its contraction dim).

v3 pipeline (per core). The hard floor is ACT exp (~33.5M elem/core at
~1 elem/cycle/lane ~= 283us); everything else is scheduled to hide under it:
  - host pre-transposes x (no on-device XBAR transpose) and pre-packs all
    weights; fp16 inputs (fp8 was tested and is too noisy for the 2e-2 gate)
  - B: q^T,k^T per head pair into a 2-deep ring (bf16); C: v (bf16, with a
    per-head ones column at index 64 for the softmax denominator)
  - D per (pair, it): scores^T = K @ Q^T as a mode-coherent region of
    (64,128)-tile matmuls, two heads on disjoint 64-row PE tiles interleaved
    A/B so they can run concurrently; exp -> bf16 es tiles; the AV region
    (128-mode, M=65 with the ones column) for it is DISPLACED to run between
    the first and second half of scores(it+1), so ACT always has queued exps
    while the PE switches modes and runs AV.
  - normalize: denominators collect at 32-aligned partition slots of per-it
    dn tiles; one batched DVE reciprocal per tile (not 1-lane [1,512] calls);
    an f32r selector-matmul broadcasts reciprocals to [128,512]; DVE muls
    normalize avT (fp16) in place.
  - E: out = avT @ w_proj + bias in fp16, emitted per-it right after the
    last pair's normalize so stores stream throughout.
"""
import sys

sys.path.insert(0, "/opt/trn_rl_repo")

import numpy as np

import concourse.bass as bass
import concourse.mybir as mybir
import concourse.tile as tile
from concourse import bacc
from concourse.bass import ts, ds

F32 = mybir.dt.float32
F32R = mybir.dt.float32r
BF16 = mybir.dt.bfloat16
FP16 = mybir.dt.float16
AF = mybir.ActivationFunctionType

SEQ = 2048
DIM = 1024
H = 8  # heads per core
HD = 64
E = 1024  # output dim
NIT = 4  # it tiles of 512
NJS = 16  # j tiles of 128
NHP = 4  # head pairs
SCALE = HD**-0.5


def build_attention(iters: int = 1, stages: int = 5):
    nc = bacc.Bacc("TRN2", target_bir_lowering=False, debug=False)
    xT = nc.dram_tensor("xT", [128, 8, SEQ], FP16, kind="ExternalInput")
    w_qk = nc.dram_tensor("w_qk", [128, 8, 1024], FP16, kind="ExternalInput")
    w_v = nc.dram_tensor("w_v", [128, 8, 512], FP16, kind="ExternalInput")
    w_proj = nc.dram_tensor("w_proj", [128, 4, E], FP16, kind="ExternalInput")
    bias = nc.dram_tensor("bias", [E], F32, kind="ExternalInput")
    out = nc.dram_tensor("out", [SEQ, E], F32, kind="ExternalOutput")

    with tile.TileContext(nc) as tc:
        with (
            tc.tile_pool(name="cpool", bufs=1) as cpool,
            tc.tile_pool(name="qkring", bufs=2) as qkring,
            tc.tile_pool(name="epool", bufs=24) as epool,
            tc.tile_pool(name="npool", bufs=8) as npool,
            tc.tile_pool(name="opool", bufs=3) as opool,
            tc.tile_pool(name="spool", bufs=2, space="PSUM") as spool,
            tc.tile_pool(name="apool", bufs=2, space="PSUM") as apool,
            tc.tile_pool(name="rpool", bufs=2, space="PSUM") as rpool,
        ):
            pools = (cpool, qkring, epool, npool, opool, spool, apool, rpool)
            if iters == 1:
                one_iter(tc, nc, xT, w_qk, w_v, w_proj, bias, out, pools, stages)
            else:
                with tc.For_i(0, iters, 1):
                    one_iter(
                        tc, nc, xT, w_qk, w_v, w_proj, bias, out, pools, stages
                    )
    nc.compile()
    return nc


def one_iter(tc, nc, xT, w_qk, w_v, w_proj, bias, out, pools, stages=5):
    cpool, qkring, epool, npool, opool, spool, apool, rpool = pools

    # ---- persistent SBUF tiles + input DMAs
    wqk_sb = cpool.tile([128, 8, 1024], FP16, tag="wqk")
    nc.sync.dma_start(wqk_sb[:], w_qk[:])
    wv_sb = cpool.tile([128, 8, 512], FP16, tag="wv")
    nc.sync.dma_start(wv_sb[:], w_v[:])
    xT_sb = cpool.tile([128, 8, SEQ], FP16, tag="xT")
    nc.sync.dma_start(xT_sb[:], xT[:])
    wproj_sb = cpool.tile([128, 4, E], FP16, tag="wproj")
    nc.sync.dma_start(wproj_sb[:], w_proj[:])
    bias_rep = cpool.tile([128, E], F32, tag="bias")
    nc.sync.dma_start(bias_rep[:], bias[None, :].to_broadcast((128, E)))

    # selector constants for the reciprocal broadcast (f32, bitcast f32r)
    sels = []
    for v in range(2):
        s_np = np.zeros((128, 128), dtype=np.float32)
        s_np[64 * v, 0:64] = 1.0
        s_np[64 * v + 32, 64:128] = 1.0
        sd = nc.inline_tensor(s_np, f"sel{v}")
        st = cpool.tile([128, 128], F32R, tag=f"sel{v}")
        nc.sync.dma_start(st[:], sd.ap().bitcast(F32R))
        sels.append(st)

    v_sb = cpool.tile([128, NJS, H * (HD + 1)], BF16, tag="v")
    nc.vector.memset(v_sb[:], 1.0)  # col 64 of each head = softmax denominator
    v_view = v_sb[:].rearrange("p j (h c) -> p j h c", c=HD + 1)

    avT = cpool.tile([128, NHP, SEQ], FP16, tag="avT")

    # ---- stage C: v = x @ w_v
    for jt2 in range(NJS // 2):
        pvs = [rpool.tile([128, 512], F32, tag="r", name=f"pv{i}") for i in range(2)]
        for ks in range(8):
            for i in range(2):
                nc.tensor.matmul(
                    pvs[i][:],
                    xT_sb[:, ks, ts(2 * jt2 + i, 128)],
                    wv_sb[:, ks, :],
                    start=(ks == 0),
                    stop=(ks == 7),
                )
        for i in range(2):
            nc.vector.tensor_copy(
                v_view[:, 2 * jt2 + i, :, 0:HD],
                pvs[i][:].rearrange("p (h c) -> p h c", c=HD),
            )

    if stages <= 1:
        out_r = out.rearrange("(p a) e -> p (a e)", p=128)
        nc.sync.dma_start(
            out_r[:].bitcast(BF16)[:, 0 : NJS * H * (HD + 1)],
            v_sb[:].rearrange("p a b -> p (a b)"),
        )
        return

    # ---- stage B: q^T,k^T for pair p into ring slot (q at [:,0,:], k at [:,1,:])
    def emit_b(p):
        qk = qkring.tile([128, 2, SEQ], BF16, tag="qk", name=f"qk{p % 2}")
        for qi, cb in ((0, p), (1, 4 + p)):
            for sb2 in range(2):
                pbs = [
                    spool.tile([128, 512], F32, tag="s", name=f"pb{i}")
                    for i in range(2)
                ]
                for ks in range(8):
                    for i in range(2):
                        nc.tensor.matmul(
                            pbs[i][:],
                            wqk_sb[:, ks, ts(cb, 128)],
                            xT_sb[:, ks, ts(2 * sb2 + i, 512)],
                            start=(ks == 0),
                            stop=(ks == 7),
                        )
                for i in range(2):
                    nc.vector.tensor_copy(
                        qk[:, qi, ts(2 * sb2 + i, 512)], pbs[i][:]
                    )
        return qk

    if stages <= 2:
        qk = emit_b(0)
        out_r = out.rearrange("(p a) e -> p (a e)", p=128)
        nc.sync.dma_start(
            out_r[:].bitcast(BF16)[:, 0 : 2 * SEQ],
            qk[:].rearrange("p a b -> p (a b)"),
        )
        return

    # dn tiles (denominators): per it, 4 pairs x 2 heads at 32-aligned slots
    # dn[it] tile 0 holds pairs 0,1 (slots 0,32 / 64,96); tile 1 pairs 2,3
    dn_tiles = {}
    for it in range(NIT):
        for t in range(2):
            d = npool.tile([128, 512], F32, tag="dn", name=f"dn{it}_{t}")
            nc.vector.memset(d[:], 1.0)
            dn_tiles[(it, t)] = d

    # ---- stage D (+ per-it normalize & E after the last pair)
    qk = emit_b(0)
    for hp in range(NHP):
        es_tiles = {}

        def emit_scores(it, jlo, jhi):
            for jsp in range(jlo, jhi):
                sp = [
                    spool.tile([128, 2, 512], F32, tag="s", name=f"sp{h01}")
                    for h01 in range(2)
                ]
                # A/B on disjoint 64-row PE tiles, interleaved for concurrency
                for half in range(2):
                    for h01 in range(2):
                        sl = slice(h01 * 64, h01 * 64 + 64)
                        nc.tensor.matmul(
                            sp[h01][:, half, :],
                            qk[sl, 1, ts(2 * jsp + half, 128)],
                            qk[sl, 0, ts(it, 512)],
                            start=True,
                            stop=True,
                        )
                for h01 in range(2):
                    e = epool.tile([128, 2, 512], BF16, tag="e")
                    nc.scalar.activation(e[:], sp[h01][:], AF.Exp)
                    es_tiles[(it, jsp, h01)] = e

        def emit_av(it):
            av_ps = [
                apool.tile([HD + 1, 512], F32, tag="av", name=f"av{h01}")
                for h01 in range(2)
            ]
            for jt in range(NJS):
                for h01 in range(2):
                    key = (it, jt // 2, h01)
                    e = es_tiles.pop(key) if jt % 2 == 1 else es_tiles[key]
                    nc.tensor.matmul(
                        av_ps[h01][:],
                        v_view[:, jt, 2 * hp + h01, :],
                        e[:, jt % 2, :],
                        start=(jt == 0),
                        stop=(jt == NJS - 1),
                    )
            # unnormalized av -> avT (fp16); denominator -> dn slot
            dnt = dn_tiles[(it, hp // 2)]
            for h01 in range(2):
                nc.vector.tensor_copy(
                    avT[ts(h01, 64), hp, ts(it, 512)], av_ps[h01][0:HD, :]
                )
                slot = 64 * (hp % 2) + 32 * h01
                nc.vector.tensor_copy(
                    dnt[slot : slot + 1, :], av_ps[h01][HD : HD + 1, :]
                )

        def emit_norm_e(it):
            rcs = []
            for t in range(2):
                rc = npool.tile([128, 512], F32R, tag="rc", name=f"rc{t}")
                # f32r keeps the f32 exponent; only low mantissa bits drop
                with nc.allow_low_precision(reason="f32r reciprocal broadcast"):
                    nc.vector.reciprocal(rc[:], dn_tiles[(it, t)][:])
                rcs.append(rc)
            for p in range(NHP):
                rr = rpool.tile([128, 512], F32, tag="r", name="rr")
                nc.tensor.matmul(
                    rr[:],
                    sels[p % 2][:],
                    rcs[p // 2][:],
                    start=True,
                    stop=True,
                )
                nc.vector.tensor_mul(
                    avT[:, p, ts(it, 512)], avT[:, p, ts(it, 512)], rr[:]
                )
            if stages >= 5:
                emit_e(nc, opool, rpool, avT, wproj_sb, bias_rep, out, it)

        # displaced pipeline: av(it) runs between the two halves of
        # scores(it+1); B(p+1) is emitted inside pair p's stream
        emit_scores(0, 0, 8)
        emit_scores(1, 0, 4)
        emit_av(0)
        if hp == NHP - 1:
            emit_norm_e(0)
        emit_scores(1, 4, 8)
        emit_scores(2, 0, 4)
        emit_av(1)
        if hp == NHP - 1:
            emit_norm_e(1)
        if hp < NHP - 1:
            next_qk = emit_b(hp + 1)
        emit_scores(2, 4, 8)
        emit_scores(3, 0, 4)
        emit_av(2)
        if hp == NHP - 1:
            emit_norm_e(2)
        emit_scores(3, 4, 8)
        emit_av(3)
        if hp == NHP - 1:
            emit_norm_e(3)
        else:
            qk = next_qk

    if stages == 4:
        out_r = out.rearrange("(p a) e -> p (a e)", p=128)
        nc.sync.dma_start(
            out_r[:, 0 : NHP * SEQ],
            avT[:].rearrange("p k s -> p (k s)").bitcast(F32),
        )
        return


def emit_e(nc, opool, rpool, avT, wproj_sb, bias_rep, out, it):
    """out[it*512 : (it+1)*512, :] = avT_slice @ w_proj + bias (fp16)."""
    for i128 in range(4):
        irow = 4 * it + i128
        pes = [
            rpool.tile([128, 512], F32, tag="r", name=f"pe{et}") for et in range(2)
        ]
        for cs in range(4):
            for et in range(2):
                nc.tensor.matmul(
                    pes[et][:],
                    avT[:, cs, ts(irow, 128)],
                    wproj_sb[:, cs, ts(et, 512)],
                    start=(cs == 0),
                    stop=(cs == 3),
                )
        for et in range(2):
            o = opool.tile([128, 512], F32, tag="o")
            nc.vector.tensor_add(o[:], pes[et][:], bias_rep[:, ts(et, 512)])
            nc.sync.dma_start(out[ts(irow, 128), ts(et, 512)], o[:])


# ---------------- host side ----------------

_CACHE = {}


def _get_runner():
    if "runner" not in _CACHE:
        import jax
        from jax.sharding import Mesh, PartitionSpec
        from jax.experimental.shard_map import shard_map
        from concourse import bass2jax

        nc = build_attention(iters=1)
        bass2jax.install_neuronx_cc_hook()

        in_names, out_names, out_avals, zero_shapes = [], [], [], []
        partition_name = nc.partition_id_tensor.name if nc.partition_id_tensor else None
        for alloc in nc.m.functions[0].allocations:
            if not isinstance(alloc, mybir.MemoryLocationSet):
                continue
            name = alloc.memorylocations[0].name
            if alloc.kind == "ExternalInput":
                if name != partition_name:
                    in_names.append(name)
            elif alloc.kind == "ExternalOutput":
                out_names.append(name)
                shape = tuple(alloc.tensor_shape)
                dtype = mybir.dt.np(alloc.dtype)
                out_avals.append(jax.core.ShapedArray(shape, dtype))
                zero_shapes.append((shape, dtype))
        n_params = len(in_names)
        n_outs = len(out_avals)
        all_names = in_names + out_names
        if partition_name is not None:
            all_names = all_names + [partition_name]
        donate = tuple(range(n_params, n_params + n_outs))

        def _body(*args):
            operands = list(args)
            if partition_name is not None:
                operands.append(bass2jax.partition_id_tensor())
            outs = bass2jax._bass_exec_p.bind(
                *operands,
                out_avals=tuple(out_avals),
                in_names=tuple(all_names),
                out_names=tuple(out_names),
                lowering_input_output_aliases=(),
                sim_require_finite=True,
                sim_require_nnan=True,
                nc=nc,
            )
            return tuple(outs)

        devices = jax.devices()[:8]
        mesh = Mesh(np.asarray(devices), ("core",))
        in_specs = (PartitionSpec("core"),) * (n_params + n_outs)
        out_specs = (PartitionSpec("core"),) * n_outs
        sharded = jax.jit(
            shard_map(
                _body,
                mesh=mesh,
                in_specs=in_specs,
                out_specs=out_specs,
                check_rep=False,
            ),
            donate_argnums=donate,
            keep_unused=True,
        )
        _CACHE["runner"] = (sharded, in_names, out_names, out_avals, zero_shapes)
    return _CACHE["runner"]


def _pack_k128(w):
    """[1024, C] -> [128, 8, C] with row = ks*128 + p."""
    return np.ascontiguousarray(w.reshape(8, 128, -1).transpose(1, 0, 2))


def _shard_inputs(x, w_qkv, w_proj, b_proj):
    """Per-core input dicts. Core c: batch c//2, head-group c%2."""
    in_maps = []
    zeros_bias = np.zeros_like(b_proj)
    for c in range(8):
        b = c // 2
        hg = c % 2
        cols = np.arange(hg * 8 * HD, (hg + 1) * 8 * HD)
        w_q = w_qkv[:, cols] * np.float32(SCALE)
        w_k = w_qkv[:, 1024 + cols]
        w_vv = w_qkv[:, 2048 + cols]
        w_qk_c = np.concatenate([w_q, w_k], axis=1)  # [1024, 1024]
        xt = np.ascontiguousarray(x[b].T)  # [1024, 2048]
        wp = w_proj[hg * 512 : (hg + 1) * 512]  # [512, 1024]
        in_maps.append(
            {
                "xT": _pack_k128(xt).astype(np.float16),
                "w_qk": _pack_k128(w_qk_c).astype(np.float16),
                "w_v": _pack_k128(w_vv).astype(np.float16),
                "w_proj": np.ascontiguousarray(
                    wp.reshape(4, 128, 1024).transpose(1, 0, 2)
                ).astype(np.float16),
                "bias": (b_proj if hg == 0 else zeros_bias).astype(np.float32),
            }
        )
    return in_maps


def kernel(x, w_qkv, w_proj, b_proj):
    import jax
    import jax.numpy as jnp

    x = np.asarray(x, dtype=np.float32)
    w_qkv = np.asarray(w_qkv, dtype=np.float32)
    w_proj = np.asarray(w_proj, dtype=np.float32)
    b_proj = np.asarray(b_proj, dtype=np.float32)

    sharded, in_names, out_names, out_avals, zero_shapes = _get_runner()
    in_maps = _shard_inputs(x, w_qkv, w_proj, b_proj)
    concat_in = [
        np.concatenate([in_maps[c][name] for c in range(8)], axis=0)
        for name in in_names
    ]
    zeros = [jnp.zeros((8 * s[0], *s[1:]), dt) for (s, dt) in zero_shapes]
    outs = sharded(*concat_in, *zeros)
    out_np = np.asarray(outs[out_names.index("out")]).reshape(8, SEQ, E)
    full = np.empty((4, SEQ, E), dtype=np.float32)
    for b in range(4):
        full[b] = out_np[2 * b] + out_np[2 * b + 1]
    return full


# revision 15
# speedup vs baseline: 27.3808x; 1.0082x over previous
"""Trainium2 Bass kernel for nn_Attention (b=4, n=2048, d=1024, 16 heads x 64).

Sharding: 8 cores = 4 batches x 2 head-groups (8 heads each); host sums the
two head-group partials per batch (proj is split along its contraction dim).

v4 pipeline (per core). Two hard facts drive the design: ACT exp is a
~280us floor (33.5M elem at ~1 elem/cycle/lane), and the PE pays a drain
every time the tile MODE changes (64-row scores vs 128-row everything else
interleave to ~430-570ns/MM).  So v4 makes EVERY matmul a (128,128)-mode
full-array op and lets the Tile scheduler interleave freely:
  - scores^T = K @ Q^T with K=128: the pair's kT (two heads stacked in the
    partition dim) is the shared stationary; the moving q of each head is
    zero-padded in the other head's 64 rows (exact, and the LDW is shared).
  - av^T = [V|1]^T @ exp^T stays K=128/M=65 (ones column -> denominator).
  - host pre-transposes x; fp16 inputs (fp8 measured too noisy); fp16 proj.
  - normalize: denominators at 32-aligned slots of per-it dn tiles, one
    batched DVE reciprocal per tile, f32r selector-matmul broadcast, DVE
    muls normalize avT (fp16) in place; E emitted per-it after the last
    pair so output stores stream throughout.
"""
import sys

sys.path.insert(0, "/opt/trn_rl_repo")

import numpy as np

import concourse.bass as bass
import concourse.mybir as mybir
import concourse.tile as tile
from concourse import bacc
from concourse.bass import ts, ds

F32 = mybir.dt.float32
F32R = mybir.dt.float32r
BF16 = mybir.dt.bfloat16
FP16 = mybir.dt.float16
AF = mybir.ActivationFunctionType

SEQ = 2048
DIM = 1024
H = 8  # heads per core
HD = 64
E = 1024  # output dim
NIT = 4  # it tiles of 512
NJS = 16  # j tiles of 128
NHP = 4  # head pairs
SCALE = HD**-0.5


def build_attention(iters: int = 1, stages: int = 5):
    nc = bacc.Bacc("TRN2", target_bir_lowering=False, debug=False)
    xT = nc.dram_tensor("xT", [128, 8, SEQ], FP16, kind="ExternalInput")
    w_qk = nc.dram_tensor("w_qk", [128, 8, 1024], FP16, kind="ExternalInput")
    w_v = nc.dram_tensor("w_v", [128, 8, 512], FP16, kind="ExternalInput")
    w_proj = nc.dram_tensor("w_proj", [128, 4, E], FP16, kind="ExternalInput")
    bias = nc.dram_tensor("bias", [E], F32, kind="ExternalInput")
    out = nc.dram_tensor("out", [SEQ, E], F32, kind="ExternalOutput")

    with tile.TileContext(nc) as tc:
        with (
            tc.tile_pool(name="cpool", bufs=1) as cpool,
            tc.tile_pool(name="qkring", bufs=2) as qkring,
            tc.tile_pool(name="epool", bufs=6) as epool,
            tc.tile_pool(name="npool", bufs=8) as npool,
            tc.tile_pool(name="opool", bufs=3) as opool,
            tc.tile_pool(name="spool", bufs=3, space="PSUM") as spool,
            tc.tile_pool(name="apool", bufs=2, space="PSUM") as apool,
        ):
            pools = (cpool, qkring, epool, npool, opool, spool, apool)
            if iters == 1:
                one_iter(tc, nc, xT, w_qk, w_v, w_proj, bias, out, pools, stages)
            else:
                with tc.For_i(0, iters, 1):
                    one_iter(
                        tc, nc, xT, w_qk, w_v, w_proj, bias, out, pools, stages
                    )
    nc.compile()
    return nc


def one_iter(tc, nc, xT, w_qk, w_v, w_proj, bias, out, pools, stages=5):
    cpool, qkring, epool, npool, opool, spool, apool = pools

    # ---- persistent SBUF tiles + input DMAs
    wqk_sb = cpool.tile([128, 8, 1024], FP16, tag="wqk")
    nc.sync.dma_start(wqk_sb[:], w_qk[:])
    wv_sb = cpool.tile([128, 8, 512], FP16, tag="wv")
    nc.sync.dma_start(wv_sb[:], w_v[:])
    xT_sb = cpool.tile([128, 8, SEQ], FP16, tag="xT")
    nc.sync.dma_start(xT_sb[:], xT[:])
    wproj_sb = cpool.tile([128, 4, E], FP16, tag="wproj")
    nc.sync.dma_start(wproj_sb[:], w_proj[:])
    bias_rep = cpool.tile([128, E], F32, tag="bias")
    nc.sync.dma_start(bias_rep[:], bias[None, :].to_broadcast((128, E)))

    # selector constants for the reciprocal broadcast (f32, bitcast f32r)
    sels = []
    for v in range(2):
        s_np = np.zeros((128, 128), dtype=np.float32)
        s_np[64 * v, 0:64] = 1.0
        s_np[64 * v + 32, 64:128] = 1.0
        sd = nc.inline_tensor(s_np, f"sel{v}")
        st = cpool.tile([128, 128], F32R, tag=f"sel{v}")
        nc.sync.dma_start(st[:], sd.ap().bitcast(F32R))
        sels.append(st)

    v_sb = cpool.tile([128, NJS, H * (HD + 1)], BF16, tag="v")
    nc.vector.memset(v_sb[:], 1.0)  # col 64 of each head = softmax denominator
    v_view = v_sb[:].rearrange("p j (h c) -> p j h c", c=HD + 1)

    avT = cpool.tile([128, NHP, SEQ], FP16, tag="avT")

    # ---- stage C: v = x @ w_v
    for jt2 in range(NJS // 2):
        pvs = [spool.tile([128, 512], F32, tag="s", name=f"pv{i}") for i in range(2)]
        for ks in range(8):
            for i in range(2):
                nc.tensor.matmul(
                    pvs[i][:],
                    xT_sb[:, ks, ts(2 * jt2 + i, 128)],
                    wv_sb[:, ks, :],
                    start=(ks == 0),
                    stop=(ks == 7),
                )
        for i in range(2):
            nc.vector.tensor_copy(
                v_view[:, 2 * jt2 + i, :, 0:HD],
                pvs[i][:].rearrange("p (h c) -> p h c", c=HD),
            )

    if stages <= 1:
        out_r = out.rearrange("(p a) e -> p (a e)", p=128)
        nc.sync.dma_start(
            out_r[:].bitcast(BF16)[:, 0 : NJS * H * (HD + 1)],
            v_sb[:].rearrange("p a b -> p (a b)"),
        )
        return

    # ---- stage B per pair p: kT (both heads stacked, [:,0,:]) and per-head
    # zero-padded q ([:,1,:] head A in rows 0-63, [:,2,:] head B in rows
    # 64-127; the other 64 rows stay zero from the one-time memset so the
    # K=128 scores matmul contracts exactly one head)
    def emit_b(p):
        qk = qkring.tile([128, 3, SEQ], BF16, tag="qk", name=f"qk{p % 2}")
        nc.vector.memset(qk[ts(1, 64), 1, :], 0.0)  # pad rows for head A's q
        nc.vector.memset(qk[ts(0, 64), 2, :], 0.0)  # pad rows for head B's q
        for qi, cb in ((0, 4 + p), (1, p)):
            for sb2 in range(2):
                pbs = [
                    spool.tile([128, 512], F32, tag="s", name=f"pb{i}")
                    for i in range(2)
                ]
                for ks in range(8):
                    for i in range(2):
                        nc.tensor.matmul(
                            pbs[i][:],
                            wqk_sb[:, ks, ts(cb, 128)],
                            xT_sb[:, ks, ts(2 * sb2 + i, 512)],
                            start=(ks == 0),
                            stop=(ks == 7),
                        )
                for i in range(2):
                    sb4 = 2 * sb2 + i
                    if qi == 0:
                        nc.vector.tensor_copy(qk[:, 0, ts(sb4, 512)], pbs[i][:])
                    else:
                        nc.vector.tensor_copy(
                            qk[ts(0, 64), 1, ts(sb4, 512)], pbs[i][ts(0, 64), :]
                        )
                        nc.vector.tensor_copy(
                            qk[ts(1, 64), 2, ts(sb4, 512)], pbs[i][ts(1, 64), :]
                        )
        return qk

    if stages <= 2:
        qk = emit_b(0)
        out_r = out.rearrange("(p a) e -> p (a e)", p=128)
        nc.sync.dma_start(
            out_r[:].bitcast(BF16)[:, 0 : 3 * SEQ],
            qk[:].rearrange("p a b -> p (a b)"),
        )
        return

    # dn tiles (denominators): per it, 4 pairs x 2 heads at 32-aligned slots
    # dn[it] tile 0 holds pairs 0,1 (slots 0,32 / 64,96); tile 1 pairs 2,3
    dn_tiles = {}
    for it in range(NIT):
        for t in range(2):
            d = npool.tile([128, 512], F32, tag="dn", name=f"dn{it}_{t}")
            nc.vector.memset(d[:], 1.0)
            dn_tiles[(it, t)] = d

    # ---- stage D (+ per-it normalize & E after the last pair)
    qk = emit_b(0)
    for hp in range(NHP):
        next_qk = None
        for it in range(NIT):
            av_ps = [
                apool.tile([HD + 1, 512], F32, tag="av", name=f"av{h01}")
                for h01 in range(2)
            ]

            def emit_av(jsp, es):
                for h01 in range(2):
                    for half in range(2):
                        nc.tensor.matmul(
                            av_ps[h01][:],
                            v_view[:, 2 * jsp + half, 2 * hp + h01, :],
                            es[h01][:, half, :],
                            start=(jsp == 0 and half == 0),
                            stop=(jsp == 7 and half == 1),
                        )

            pend = []
            for jsp in range(8):
                cur = []
                for h01 in range(2):
                    sp = spool.tile([128, 2, 512], F32, tag="s", name=f"sp{h01}")
                    for half in range(2):
                        nc.tensor.matmul(
                            sp[:, half, :],
                            qk[:, 0, ts(2 * jsp + half, 128)],
                            qk[:, 1 + h01, ts(it, 512)],
                            start=True,
                            stop=True,
                        )
                    e = epool.tile([128, 2, 512], BF16, tag="e")
                    nc.scalar.activation(e[:], sp[:], AF.Exp)
                    cur.append(e)
                pend.append((jsp, cur))
                if len(pend) > 2:
                    emit_av(*pend.pop(0))
            for jsp, cur in pend:
                emit_av(jsp, cur)

            # unnormalized av -> avT (fp16); denominator -> dn slot
            dnt = dn_tiles[(it, hp // 2)]
            for h01 in range(2):
                nc.vector.tensor_copy(
                    avT[ts(h01, 64), hp, ts(it, 512)], av_ps[h01][0:HD, :]
                )
                slot = 64 * (hp % 2) + 32 * h01
                nc.vector.tensor_copy(
                    dnt[slot : slot + 1, :], av_ps[h01][HD : HD + 1, :]
                )

            if hp == NHP - 1:
                rcs = []
                for t in range(2):
                    rc = npool.tile([128, 512], F32R, tag="rc", name=f"rc{t}")
                    with nc.allow_low_precision(reason="f32r recip broadcast"):
                        nc.vector.reciprocal(rc[:], dn_tiles[(it, t)][:])
                    rcs.append(rc)
                for p in range(NHP):
                    rr = spool.tile([128, 512], F32, tag="s", name="rr")
                    nc.tensor.matmul(
                        rr[:], sels[p % 2][:], rcs[p // 2][:],
                        start=True, stop=True,
                    )
                    nc.vector.tensor_mul(
                        avT[:, p, ts(it, 512)], avT[:, p, ts(it, 512)], rr[:]
                    )
                if stages >= 5:
                    emit_e(nc, opool, spool, avT, wproj_sb, bias_rep, out, it)
            elif it == 1 and next_qk is None:
                next_qk = emit_b(hp + 1)
        if hp < NHP - 1:
            qk = next_qk

    if stages == 4:
        out_r = out.rearrange("(p a) e -> p (a e)", p=128)
        nc.sync.dma_start(
            out_r[:, 0 : NHP * SEQ],
            avT[:].rearrange("p k s -> p (k s)").bitcast(F32),
        )
        return


def emit_e(nc, opool, spool, avT, wproj_sb, bias_rep, out, it):
    """out[it*512 : (it+1)*512, :] = avT_slice @ w_proj + bias (fp16)."""
    for i128 in range(4):
        irow = 4 * it + i128
        pes = [
            spool.tile([128, 512], F32, tag="s", name=f"pe{et}") for et in range(2)
        ]
        for cs in range(4):
            for et in range(2):
                nc.tensor.matmul(
                    pes[et][:],
                    avT[:, cs, ts(irow, 128)],
                    wproj_sb[:, cs, ts(et, 512)],
                    start=(cs == 0),
                    stop=(cs == 3),
                )
        for et in range(2):
            o = opool.tile([128, 512], F32, tag="o")
            nc.vector.tensor_add(o[:], pes[et][:], bias_rep[:, ts(et, 512)])
            nc.sync.dma_start(out[ts(irow, 128), ts(et, 512)], o[:])


# ---------------- host side ----------------

_CACHE = {}


def _get_runner():
    if "runner" not in _CACHE:
        import jax
        from jax.sharding import Mesh, PartitionSpec
        from jax.experimental.shard_map import shard_map
        from concourse import bass2jax

        nc = build_attention(iters=1)
        bass2jax.install_neuronx_cc_hook()

        in_names, out_names, out_avals, zero_shapes = [], [], [], []
        partition_name = nc.partition_id_tensor.name if nc.partition_id_tensor else None
        for alloc in nc.m.functions[0].allocations:
            if not isinstance(alloc, mybir.MemoryLocationSet):
                continue
            name = alloc.memorylocations[0].name
            if alloc.kind == "ExternalInput":
                if name != partition_name:
                    in_names.append(name)
            elif alloc.kind == "ExternalOutput":
                out_names.append(name)
                shape = tuple(alloc.tensor_shape)
                dtype = mybir.dt.np(alloc.dtype)
                out_avals.append(jax.core.ShapedArray(shape, dtype))
                zero_shapes.append((shape, dtype))
        n_params = len(in_names)
        n_outs = len(out_avals)
        all_names = in_names + out_names
        if partition_name is not None:
            all_names = all_names + [partition_name]
        donate = tuple(range(n_params, n_params + n_outs))

        def _body(*args):
            operands = list(args)
            if partition_name is not None:
                operands.append(bass2jax.partition_id_tensor())
            outs = bass2jax._bass_exec_p.bind(
                *operands,
                out_avals=tuple(out_avals),
                in_names=tuple(all_names),
                out_names=tuple(out_names),
                lowering_input_output_aliases=(),
                sim_require_finite=True,
                sim_require_nnan=True,
                nc=nc,
            )
            return tuple(outs)

        devices = jax.devices()[:8]
        mesh = Mesh(np.asarray(devices), ("core",))
        in_specs = (PartitionSpec("core"),) * (n_params + n_outs)
        out_specs = (PartitionSpec("core"),) * n_outs
        sharded = jax.jit(
            shard_map(
                _body,
                mesh=mesh,
                in_specs=in_specs,
                out_specs=out_specs,
                check_rep=False,
            ),
            donate_argnums=donate,
            keep_unused=True,
        )
        _CACHE["runner"] = (sharded, in_names, out_names, out_avals, zero_shapes)
    return _CACHE["runner"]


def _pack_k128(w):
    """[1024, C] -> [128, 8, C] with row = ks*128 + p."""
    return np.ascontiguousarray(w.reshape(8, 128, -1).transpose(1, 0, 2))


def _shard_inputs(x, w_qkv, w_proj, b_proj):
    """Per-core input dicts. Core c: batch c//2, head-group c%2."""
    in_maps = []
    zeros_bias = np.zeros_like(b_proj)
    for c in range(8):
        b = c // 2
        hg = c % 2
        cols = np.arange(hg * 8 * HD, (hg + 1) * 8 * HD)
        w_q = w_qkv[:, cols] * np.float32(SCALE)
        w_k = w_qkv[:, 1024 + cols]
        w_vv = w_qkv[:, 2048 + cols]
        w_qk_c = np.concatenate([w_q, w_k], axis=1)  # [1024, 1024]
        xt = np.ascontiguousarray(x[b].T)  # [1024, 2048]
        wp = w_proj[hg * 512 : (hg + 1) * 512]  # [512, 1024]
        in_maps.append(
            {
                "xT": _pack_k128(xt).astype(np.float16),
                "w_qk": _pack_k128(w_qk_c).astype(np.float16),
                "w_v": _pack_k128(w_vv).astype(np.float16),
                "w_proj": np.ascontiguousarray(
                    wp.reshape(4, 128, 1024).transpose(1, 0, 2)
                ).astype(np.float16),
                "bias": (b_proj if hg == 0 else zeros_bias).astype(np.float32),
            }
        )
    return in_maps


def kernel(x, w_qkv, w_proj, b_proj):
    import jax
    import jax.numpy as jnp

    x = np.asarray(x, dtype=np.float32)
    w_qkv = np.asarray(w_qkv, dtype=np.float32)
    w_proj = np.asarray(w_proj, dtype=np.float32)
    b_proj = np.asarray(b_proj, dtype=np.float32)

    sharded, in_names, out_names, out_avals, zero_shapes = _get_runner()
    in_maps = _shard_inputs(x, w_qkv, w_proj, b_proj)
    concat_in = [
        np.concatenate([in_maps[c][name] for c in range(8)], axis=0)
        for name in in_names
    ]
    zeros = [jnp.zeros((8 * s[0], *s[1:]), dt) for (s, dt) in zero_shapes]
    outs = sharded(*concat_in, *zeros)
    out_np = np.asarray(outs[out_names.index("out")]).reshape(8, SEQ, E)
    full = np.empty((4, SEQ, E), dtype=np.float32)
    for b in range(4):
        full[b] = out_np[2 * b] + out_np[2 * b + 1]
    return full


# revision 17
# speedup vs baseline: 34.9527x; 1.2765x over previous
"""Trainium2 Bass kernel for nn_Attention (b=4, n=2048, d=1024, 16 heads x 64).

Sharding: 8 cores = 4 batches x 2 head-groups (8 heads each); host sums the
two head-group partials per batch (proj is split along its contraction dim).

v4 pipeline (per core). Two hard facts drive the design: ACT exp is a
~280us floor (33.5M elem at ~1 elem/cycle/lane), and the PE pays a drain
every time the tile MODE changes (64-row scores vs 128-row everything else
interleave to ~430-570ns/MM).  So v4 makes EVERY matmul a (128,128)-mode
full-array op and lets the Tile scheduler interleave freely:
  - scores^T = K @ Q^T with K=128: the pair's kT (two heads stacked in the
    partition dim) is the shared stationary; the moving q of each head is
    zero-padded in the other head's 64 rows (exact, and the LDW is shared).
  - av^T = [V|1]^T @ exp^T stays K=128/M=65 (ones column -> denominator).
  - host pre-transposes x; fp16 inputs (fp8 measured too noisy); fp16 proj.
  - normalize: denominators at 32-aligned slots of per-it dn tiles, one
    batched DVE reciprocal per tile, f32r selector-matmul broadcast, DVE
    muls normalize avT (fp16) in place; E emitted per-it after the last
    pair so output stores stream throughout.
"""
import sys

sys.path.insert(0, "/opt/trn_rl_repo")

import numpy as np

import concourse.bass as bass
import concourse.mybir as mybir
import concourse.tile as tile
from concourse import bacc
from concourse.bass import ts, ds

F32 = mybir.dt.float32
F32R = mybir.dt.float32r
BF16 = mybir.dt.bfloat16
FP16 = mybir.dt.float16
AF = mybir.ActivationFunctionType

SEQ = 2048
DIM = 1024
H = 8  # heads per core
HD = 64
E = 1024  # output dim
NIT = 4  # it tiles of 512
NJS = 16  # j tiles of 128
NHP = 4  # head pairs
SCALE = HD**-0.5


def build_attention(iters: int = 1, stages: int = 5):
    nc = bacc.Bacc("TRN2", target_bir_lowering=False, debug=False)
    xT = nc.dram_tensor("xT", [128, 8, SEQ], FP16, kind="ExternalInput")
    w_qk = nc.dram_tensor("w_qk", [128, 8, 1024], FP16, kind="ExternalInput")
    w_v = nc.dram_tensor("w_v", [128, 8, 512], FP16, kind="ExternalInput")
    w_proj = nc.dram_tensor("w_proj", [128, 4, E], FP16, kind="ExternalInput")
    bias = nc.dram_tensor("bias", [E], F32, kind="ExternalInput")
    out = nc.dram_tensor("out", [SEQ, E], F32, kind="ExternalOutput")

    with tile.TileContext(nc) as tc:
        with (
            tc.tile_pool(name="cpool", bufs=1) as cpool,
            tc.tile_pool(name="qkring", bufs=2) as qkring,
            tc.tile_pool(name="epool", bufs=6) as epool,
            tc.tile_pool(name="npool", bufs=8) as npool,
            tc.tile_pool(name="opool", bufs=3) as opool,
            tc.tile_pool(name="spool", bufs=2, space="PSUM") as spool,
            tc.tile_pool(name="rpool", bufs=2, space="PSUM") as rpool,
            tc.tile_pool(name="apool", bufs=2, space="PSUM") as apool,
        ):
            pools = (cpool, qkring, epool, npool, opool, spool, apool, rpool)
            if iters == 1:
                one_iter(tc, nc, xT, w_qk, w_v, w_proj, bias, out, pools, stages)
            else:
                with tc.For_i(0, iters, 1):
                    one_iter(
                        tc, nc, xT, w_qk, w_v, w_proj, bias, out, pools, stages
                    )
    nc.compile()
    return nc


def one_iter(tc, nc, xT, w_qk, w_v, w_proj, bias, out, pools, stages=5):
    cpool, qkring, epool, npool, opool, spool, apool, rpool = pools

    # ---- persistent SBUF tiles + input DMAs
    wqk_sb = cpool.tile([128, 8, 1024], FP16, tag="wqk")
    nc.sync.dma_start(wqk_sb[:], w_qk[:])
    wv_sb = cpool.tile([128, 8, 512], FP16, tag="wv")
    nc.sync.dma_start(wv_sb[:], w_v[:])
    xT_sb = cpool.tile([128, 8, SEQ], FP16, tag="xT")
    nc.sync.dma_start(xT_sb[:], xT[:])
    wproj_sb = cpool.tile([128, 4, E], FP16, tag="wproj")
    nc.sync.dma_start(wproj_sb[:], w_proj[:])
    bias_rep = cpool.tile([128, E], F32, tag="bias")
    nc.sync.dma_start(bias_rep[:], bias[None, :].to_broadcast((128, E)))

    # selector constants for the reciprocal broadcast (f32, bitcast f32r)
    sels = []
    for v in range(2):
        s_np = np.zeros((128, 128), dtype=np.float32)
        s_np[64 * v, 0:64] = 1.0
        s_np[64 * v + 32, 64:128] = 1.0
        sd = nc.inline_tensor(s_np, f"sel{v}")
        st = cpool.tile([128, 128], F32R, tag=f"sel{v}")
        nc.sync.dma_start(st[:], sd.ap().bitcast(F32R))
        sels.append(st)

    v_sb = cpool.tile([128, NJS, H * (HD + 1)], BF16, tag="v")
    nc.vector.memset(v_sb[:], 1.0)  # col 64 of each head = softmax denominator
    v_view = v_sb[:].rearrange("p j (h c) -> p j h c", c=HD + 1)

    avT = cpool.tile([128, NHP, SEQ], FP16, tag="avT")

    # two persistent qk slots (pairs alternate); zero padding written once
    qk_slots = [
        cpool.tile([128, 3, SEQ], BF16, tag=f"qks{i}", name=f"qks{i}")
        for i in range(2)
    ]
    for qs in qk_slots:
        nc.vector.memset(qs[ts(1, 64), 1, :], 0.0)  # pad rows for head A's q
        nc.vector.memset(qs[ts(0, 64), 2, :], 0.0)  # pad rows for head B's q

    # ---- stage C: v = x @ w_v
    for jt2 in range(NJS // 2):
        pvs = [rpool.tile([128, 512], F32, tag="r", name=f"pv{i}") for i in range(2)]
        for ks in range(8):
            for i in range(2):
                nc.tensor.matmul(
                    pvs[i][:],
                    xT_sb[:, ks, ts(2 * jt2 + i, 128)],
                    wv_sb[:, ks, :],
                    start=(ks == 0),
                    stop=(ks == 7),
                )
        for i in range(2):
            nc.vector.tensor_copy(
                v_view[:, 2 * jt2 + i, :, 0:HD],
                pvs[i][:].rearrange("p (h c) -> p h c", c=HD),
            )

    if stages <= 1:
        out_r = out.rearrange("(p a) e -> p (a e)", p=128)
        nc.sync.dma_start(
            out_r[:].bitcast(BF16)[:, 0 : NJS * H * (HD + 1)],
            v_sb[:].rearrange("p a b -> p (a b)"),
        )
        return

    # ---- stage B per pair p: kT (both heads stacked, [:,0,:]) and per-head
    # zero-padded q ([:,1,:] head A in rows 0-63, [:,2,:] head B in rows
    # 64-127; the other 64 rows stay zero from the one-time memset so the
    # K=128 scores matmul contracts exactly one head)
    def emit_b(p):
        qk = qk_slots[p % 2]
        for qi, cb in ((0, 4 + p), (1, p)):
            for sb2 in range(2):
                pbs = [
                    rpool.tile([128, 512], F32, tag="r", name=f"pb{i}")
                    for i in range(2)
                ]
                for ks in range(8):
                    for i in range(2):
                        nc.tensor.matmul(
                            pbs[i][:],
                            wqk_sb[:, ks, ts(cb, 128)],
                            xT_sb[:, ks, ts(2 * sb2 + i, 512)],
                            start=(ks == 0),
                            stop=(ks == 7),
                        )
                for i in range(2):
                    sb4 = 2 * sb2 + i
                    if qi == 0:
                        nc.vector.tensor_copy(qk[:, 0, ts(sb4, 512)], pbs[i][:])
                    else:
                        nc.vector.tensor_copy(
                            qk[ts(0, 64), 1, ts(sb4, 512)], pbs[i][ts(0, 64), :]
                        )
                        nc.vector.tensor_copy(
                            qk[ts(1, 64), 2, ts(sb4, 512)], pbs[i][ts(1, 64), :]
                        )
        return qk

    if stages <= 2:
        qk = emit_b(0)
        out_r = out.rearrange("(p a) e -> p (a e)", p=128)
        nc.sync.dma_start(
            out_r[:].bitcast(BF16)[:, 0 : 3 * SEQ],
            qk[:].rearrange("p a b -> p (a b)"),
        )
        return

    # dn tiles (denominators): per it, 4 pairs x 2 heads at 32-aligned slots
    # dn[it] tile 0 holds pairs 0,1 (slots 0,32 / 64,96); tile 1 pairs 2,3
    dn_tiles = {}
    for it in range(NIT):
        for t in range(2):
            d = npool.tile([128, 512], F32, tag="dn", name=f"dn{it}_{t}")
            nc.vector.memset(d[:], 1.0)
            dn_tiles[(it, t)] = d

    # ---- stage D (+ per-it normalize & E after the last pair)
    qk = emit_b(0)
    for hp in range(NHP):
        next_qk = None
        for it in range(NIT):
            av_ps = [
                apool.tile([HD + 1, 512], F32, tag="av", name=f"av{h01}")
                for h01 in range(2)
            ]

            def emit_av(jsp, es):
                for h01 in range(2):
                    for half in range(2):
                        nc.tensor.matmul(
                            av_ps[h01][:],
                            v_view[:, 2 * jsp + half, 2 * hp + h01, :],
                            es[h01][:, half, :],
                            start=(jsp == 0 and half == 0),
                            stop=(jsp == 7 and half == 1),
                        )

            pend = []
            for jsp in range(8):
                cur = []
                for h01 in range(2):
                    sp = spool.tile([128, 2, 512], F32, tag="s", name=f"sp{h01}")
                    for half in range(2):
                        nc.tensor.matmul(
                            sp[:, half, :],
                            qk[:, 0, ts(2 * jsp + half, 128)],
                            qk[:, 1 + h01, ts(it, 512)],
                            start=True,
                            stop=True,
                        )
                    e = epool.tile([128, 2, 512], BF16, tag="e")
                    nc.scalar.activation(e[:], sp[:], AF.Exp)
                    cur.append(e)
                pend.append((jsp, cur))
                if len(pend) > 2:
                    emit_av(*pend.pop(0))
            for jsp, cur in pend:
                emit_av(jsp, cur)

            # unnormalized av -> avT (fp16); denominator -> dn slot
            dnt = dn_tiles[(it, hp // 2)]
            for h01 in range(2):
                nc.vector.tensor_copy(
                    avT[ts(h01, 64), hp, ts(it, 512)], av_ps[h01][0:HD, :]
                )
                slot = 64 * (hp % 2) + 32 * h01
                nc.vector.tensor_copy(
                    dnt[slot : slot + 1, :], av_ps[h01][HD : HD + 1, :]
                )

            if hp == NHP - 1:
                rcs = []
                for t in range(2):
                    rc = npool.tile([128, 512], F32R, tag="rc", name=f"rc{t}")
                    with nc.allow_low_precision(reason="f32r recip broadcast"):
                        nc.vector.reciprocal(rc[:], dn_tiles[(it, t)][:])
                    rcs.append(rc)
                for p in range(NHP):
                    rr = rpool.tile([128, 512], F32, tag="r", name="rr")
                    nc.tensor.matmul(
                        rr[:], sels[p % 2][:], rcs[p // 2][:],
                        start=True, stop=True,
                    )
                    nc.vector.tensor_mul(
                        avT[:, p, ts(it, 512)], avT[:, p, ts(it, 512)], rr[:]
                    )
                if stages >= 5:
                    emit_e(nc, opool, rpool, avT, wproj_sb, bias_rep, out, it)
            elif it == 1 and next_qk is None:
                next_qk = emit_b(hp + 1)
        if hp < NHP - 1:
            qk = next_qk

    if stages == 4:
        out_r = out.rearrange("(p a) e -> p (a e)", p=128)
        nc.sync.dma_start(
            out_r[:, 0 : NHP * SEQ],
            avT[:].rearrange("p k s -> p (k s)").bitcast(F32),
        )
        return


def emit_e(nc, opool, rpool, avT, wproj_sb, bias_rep, out, it):
    """out[it*512 : (it+1)*512, :] = avT_slice @ w_proj + bias (fp16)."""
    for i128 in range(4):
        irow = 4 * it + i128
        pes = [
            rpool.tile([128, 512], F32, tag="r", name=f"pe{et}") for et in range(2)
        ]
        for cs in range(4):
            for et in range(2):
                nc.tensor.matmul(
                    pes[et][:],
                    avT[:, cs, ts(irow, 128)],
                    wproj_sb[:, cs, ts(et, 512)],
                    start=(cs == 0),
                    stop=(cs == 3),
                )
        for et in range(2):
            o = opool.tile([128, 512], F32, tag="o")
            nc.vector.tensor_add(o[:], pes[et][:], bias_rep[:, ts(et, 512)])
            nc.sync.dma_start(out[ts(irow, 128), ts(et, 512)], o[:])


# ---------------- host side ----------------

_CACHE = {}


def _get_runner():
    if "runner" not in _CACHE:
        import jax
        from jax.sharding import Mesh, PartitionSpec
        from jax.experimental.shard_map import shard_map
        from concourse import bass2jax

        nc = build_attention(iters=1)
        bass2jax.install_neuronx_cc_hook()

        in_names, out_names, out_avals, zero_shapes = [], [], [], []
        partition_name = nc.partition_id_tensor.name if nc.partition_id_tensor else None
        for alloc in nc.m.functions[0].allocations:
            if not isinstance(alloc, mybir.MemoryLocationSet):
                continue
            name = alloc.memorylocations[0].name
            if alloc.kind == "ExternalInput":
                if name != partition_name:
                    in_names.append(name)
            elif alloc.kind == "ExternalOutput":
                out_names.append(name)
                shape = tuple(alloc.tensor_shape)
                dtype = mybir.dt.np(alloc.dtype)
                out_avals.append(jax.core.ShapedArray(shape, dtype))
                zero_shapes.append((shape, dtype))
        n_params = len(in_names)
        n_outs = len(out_avals)
        all_names = in_names + out_names
        if partition_name is not None:
            all_names = all_names + [partition_name]
        donate = tuple(range(n_params, n_params + n_outs))

        def _body(*args):
            operands = list(args)
            if partition_name is not None:
                operands.append(bass2jax.partition_id_tensor())
            outs = bass2jax._bass_exec_p.bind(
                *operands,
                out_avals=tuple(out_avals),
                in_names=tuple(all_names),
                out_names=tuple(out_names),
                lowering_input_output_aliases=(),
                sim_require_finite=True,
                sim_require_nnan=True,
                nc=nc,
            )
            return tuple(outs)

        devices = jax.devices()[:8]
        mesh = Mesh(np.asarray(devices), ("core",))
        in_specs = (PartitionSpec("core"),) * (n_params + n_outs)
        out_specs = (PartitionSpec("core"),) * n_outs
        sharded = jax.jit(
            shard_map(
                _body,
                mesh=mesh,
                in_specs=in_specs,
                out_specs=out_specs,
                check_rep=False,
            ),
            donate_argnums=donate,
            keep_unused=True,
        )
        _CACHE["runner"] = (sharded, in_names, out_names, out_avals, zero_shapes)
    return _CACHE["runner"]


def _pack_k128(w):
    """[1024, C] -> [128, 8, C] with row = ks*128 + p."""
    return np.ascontiguousarray(w.reshape(8, 128, -1).transpose(1, 0, 2))


def _shard_inputs(x, w_qkv, w_proj, b_proj):
    """Per-core input dicts. Core c: batch c//2, head-group c%2."""
    in_maps = []
    zeros_bias = np.zeros_like(b_proj)
    for c in range(8):
        b = c // 2
        hg = c % 2
        cols = np.arange(hg * 8 * HD, (hg + 1) * 8 * HD)
        w_q = w_qkv[:, cols] * np.float32(SCALE)
        w_k = w_qkv[:, 1024 + cols]
        w_vv = w_qkv[:, 2048 + cols]
        w_qk_c = np.concatenate([w_q, w_k], axis=1)  # [1024, 1024]
        xt = np.ascontiguousarray(x[b].T)  # [1024, 2048]
        wp = w_proj[hg * 512 : (hg + 1) * 512]  # [512, 1024]
        in_maps.append(
            {
                "xT": _pack_k128(xt).astype(np.float16),
                "w_qk": _pack_k128(w_qk_c).astype(np.float16),
                "w_v": _pack_k128(w_vv).astype(np.float16),
                "w_proj": np.ascontiguousarray(
                    wp.reshape(4, 128, 1024).transpose(1, 0, 2)
                ).astype(np.float16),
                "bias": (b_proj if hg == 0 else zeros_bias).astype(np.float32),
            }
        )
    return in_maps


def kernel(x, w_qkv, w_proj, b_proj):
    import jax
    import jax.numpy as jnp

    x = np.asarray(x, dtype=np.float32)
    w_qkv = np.asarray(w_qkv, dtype=np.float32)
    w_proj = np.asarray(w_proj, dtype=np.float32)
    b_proj = np.asarray(b_proj, dtype=np.float32)

    sharded, in_names, out_names, out_avals, zero_shapes = _get_runner()
    in_maps = _shard_inputs(x, w_qkv, w_proj, b_proj)
    concat_in = [
        np.concatenate([in_maps[c][name] for c in range(8)], axis=0)
        for name in in_names
    ]
    zeros = [jnp.zeros((8 * s[0], *s[1:]), dt) for (s, dt) in zero_shapes]
    outs = sharded(*concat_in, *zeros)
    out_np = np.asarray(outs[out_names.index("out")]).reshape(8, SEQ, E)
    full = np.empty((4, SEQ, E), dtype=np.float32)
    for b in range(4):
        full[b] = out_np[2 * b] + out_np[2 * b + 1]
    return full
